# revision 5
# baseline (speedup 1.0000x reference)
"""KmeansVectorQuantizer forward on 8 Trainium2 NeuronCores.

Problem (hardcoded): B=4, T=2048, G=8, D=128, V=1024, BETA=1.0.
Sharding: data-parallel over flattened (B*T)=8192 tokens, 1024/core;
codebook replicated. Losses via per-core partial sums, finished on host.

Device pipeline per core:
  - PE transposes codebook group-blocks -> C^T_g [d, v] (once) and
    x tiles -> X^T_g [d, t].
  - fp32r matmuls: xc[t, v] = X_g^T.T @ C_g^T  (K=d=128, N=512 x2).
  - DVE: negdist = xc - c2half_g (c2half host-precomputed, broadcast),
    max8 + max_index -> argmin ids.
  - GPSIMD indirect DMA gathers codebook rows -> quantized; ACT scales by
    mask (zeroes padded tokens).
  - Loss identity: ||x - c_id||^2 = ||x||^2 - 2*max(negdist); summed per
    partition, host does the final reduction and division.
"""

import numpy as np

B, T, G, D, V = 4, 2048, 8, 128, 1024
GD = G * D
N_CORES = 8
TOK = B * T // N_CORES      # tokens per core
NT = TOK // 128             # token tiles per core

_cache = {}


def _build_program():
    import concourse.bass as bass
    import concourse.bacc as bacc
    import concourse.mybir as mybir
    import concourse.tile as tile
    from concourse.masks import make_identity

    f32 = mybir.dt.float32
    u32 = mybir.dt.uint32

    nc = bacc.Bacc(
        "TRN2",
        target_bir_lowering=False,
        debug=False,
        num_devices=N_CORES,
    )

    x_d = nc.dram_tensor("x", [TOK, GD], f32, kind="ExternalInput").ap()
    cb_d = nc.dram_tensor("cb", [V, GD], f32, kind="ExternalInput").ap()
    c2hb_d = nc.dram_tensor("c2hb", [G * 128, V], f32, kind="ExternalInput").ap()
    mask_d = nc.dram_tensor("mask", [TOK, 1], f32, kind="ExternalInput").ap()

    ids_d = nc.dram_tensor("ids_u", [TOK, G], u32, kind="ExternalOutput").ap()
    q_d = nc.dram_tensor("q", [TOK, GD], f32, kind="ExternalOutput").ap()
    lp_d = nc.dram_tensor("lpart", [128, 1], f32, kind="ExternalOutput").ap()

    with tile.TileContext(nc) as tc:
        with (
            tc.tile_pool(name="const", bufs=1) as constp,
            tc.tile_pool(name="cbn", bufs=2) as cbnp,
            tc.tile_pool(name="xt", bufs=2) as xtp,
            tc.tile_pool(name="xT", bufs=3) as xTp,
            tc.tile_pool(name="nd", bufs=2) as ndp,
            tc.tile_pool(name="qt", bufs=2) as qtp,
            tc.tile_pool(name="tiny", bufs=4) as tinyp,
            tc.tile_pool(name="scr", bufs=2) as scrp,
            tc.tile_pool(name="psT", bufs=2, space="PSUM") as psT,
            tc.tile_pool(name="psMM", bufs=3, space="PSUM") as psMM,
        ):
            ident = constp.tile([128, 128], f32, tag="ident")
            make_identity(nc, ident[:])

            # Replicated codebook transpose: ct[g] = C_g^T  [d=128, v=1024]
            ct = [constp.tile([128, V], f32, tag=f"ct{g}", name=f"ct{g}") for g in range(G)]
            for vt in range(V // 128):
                cbn = cbnp.tile([128, GD], f32, tag="cbn")
                nc.sync.dma_start(cbn[:], cb_d[vt * 128:(vt + 1) * 128, :])
                for g in range(G):
                    tp = psT.tile([128, 128], f32, tag="tp")
                    nc.tensor.transpose(
                        tp[:], cbn[:, g * 128:(g + 1) * 128], ident[:]
                    )
                    nc.vector.tensor_copy(
                        ct[g][:, vt * 128:(vt + 1) * 128], tp[:]
                    )

            # Host-precomputed 0.5*||c||^2, broadcast to 128 partitions per g
            c2b = [constp.tile([128, V], f32, tag=f"c2b{g}", name=f"c2b{g}") for g in range(G)]
            for g in range(G):
                nc.sync.dma_start(
                    c2b[g][:], c2hb_d[g * 128:(g + 1) * 128, :]
                )

            lacc = constp.tile([128, 1], f32, tag="lacc")
            nc.vector.memset(lacc[:], 0.0)

            for tt in range(NT):
                xt = xtp.tile([128, GD], f32, tag="xt")
                nc.sync.dma_start(xt[:], x_d[tt * 128:(tt + 1) * 128, :])
                mk = tinyp.tile([128, 1], f32, tag="mk")
                nc.sync.dma_start(mk[:], mask_d[tt * 128:(tt + 1) * 128, :])

                # x2_tot[t] = sum_d x^2 over all groups
                scr = scrp.tile([128, GD], f32, tag="scr")
                x2t = tinyp.tile([128, 1], f32, tag="x2t")
                nc.scalar.activation(
                    out=scr[:], in_=xt[:],
                    func=mybir.ActivationFunctionType.Square,
                    accum_out=x2t[:],
                )

                qt = qtp.tile([128, GD], f32, tag="qt")
                idsv = tinyp.tile([128, G], u32, tag="idsv")
                maxv = tinyp.tile([128, G], f32, tag="maxv")

                for g in range(G):
                    tp = psT.tile([128, 128], f32, tag="tp")
                    nc.tensor.transpose(
                        tp[:], xt[:, g * 128:(g + 1) * 128], ident[:]
                    )
                    xT = xTp.tile([128, 128], f32, tag="xT")
                    nc.vector.tensor_copy(xT[:], tp[:])

                    mm = psMM.tile([128, V], f32, tag="mm")
                    for h in range(2):
                        nc.tensor.matmul(
                            mm[:, h * 512:(h + 1) * 512],
                            lhsT=xT[:],
                            rhs=ct[g][:, h * 512:(h + 1) * 512],
                            start=True,
                            stop=True,
                        )

                    nd = ndp.tile([128, V], f32, tag="nd")
                    nc.vector.tensor_sub(nd[:], mm[:], c2b[g][:])

                    mx8 = tinyp.tile([128, 8], f32, tag="mx8")
                    nc.vector.max(mx8[:], nd[:])
                    mi8 = tinyp.tile([128, 8], u32, tag="mi8")
                    nc.vector.max_index(mi8[:], mx8[:], nd[:])

                    nc.vector.tensor_copy(maxv[:, g:g + 1], mx8[:, 0:1])
                    nc.vector.tensor_copy(idsv[:, g:g + 1], mi8[:, 0:1])

                    import concourse.bass as bass_mod
                    nc.gpsimd.indirect_dma_start(
                        out=qt[:, g * 128:(g + 1) * 128],
                        out_offset=None,
                        in_=cb_d[:, :],
                        in_offset=bass_mod.IndirectOffsetOnAxis(
                            ap=mi8[:, 0:1], axis=0
                        ),
                        element_offset=g * 128,
                    )

                nc.sync.dma_start(ids_d[tt * 128:(tt + 1) * 128, :], idsv[:])

                qm = qtp.tile([128, GD], f32, tag="qm")
                nc.scalar.activation(
                    out=qm[:], in_=qt[:],
                    func=mybir.ActivationFunctionType.Copy,
                    scale=mk[:, 0:1],
                )
                nc.sync.dma_start(q_d[tt * 128:(tt + 1) * 128, :], qm[:])

                # loss partial: (x2_tot - 2*sum_g maxv) * mask, accumulated
                ds = tinyp.tile([128, 1], f32, tag="ds")
                nc.vector.reduce_sum(ds[:], maxv[:], axis=mybir.AxisListType.X)
                t1 = tinyp.tile([128, 1], f32, tag="t1")
                nc.vector.tensor_scalar(
                    t1[:], ds[:], -2.0, x2t[:, 0:1],
                    op0=mybir.AluOpType.mult, op1=mybir.AluOpType.add,
                )
                t2 = tinyp.tile([128, 1], f32, tag="t2")
                nc.vector.tensor_scalar(
                    t2[:], t1[:], mk[:, 0:1], None,
                    op0=mybir.AluOpType.mult,
                )
                nc.vector.tensor_add(lacc[:], lacc[:], t2[:])

            nc.sync.dma_start(lp_d[:, :], lacc[:])

    nc.compile()
    return nc


def _get_program():
    if "nc" not in _cache:
        _cache["nc"] = _build_program()
    return _cache["nc"]


def kernel(inputs: np.ndarray, paddings: np.ndarray, codebook: np.ndarray):
    from concourse import bass_utils

    x = np.ascontiguousarray(inputs.reshape(B * T, GD), dtype=np.float32)
    pad = np.asarray(paddings).reshape(B * T)
    cb = np.ascontiguousarray(codebook.reshape(V, GD), dtype=np.float32)
    mask = (1 - pad).astype(np.float32).reshape(B * T, 1)

    # 0.5*||c_vg||^2 in f64 then f32, pre-broadcast to [G*128, V]
    c2h = 0.5 * (codebook.astype(np.float64) ** 2).sum(axis=2)   # (V, G)
    c2h = np.ascontiguousarray(c2h.T, dtype=np.float32)           # (G, V)
    c2hb = np.ascontiguousarray(
        np.repeat(c2h[:, None, :], 128, axis=1).reshape(G * 128, V)
    )

    in_maps = []
    for c in range(N_CORES):
        sl = slice(c * TOK, (c + 1) * TOK)
        in_maps.append({
            "x": x[sl],
            "cb": cb,
            "c2hb": c2hb,
            "mask": mask[sl],
        })

    nc = _get_program()
    res = bass_utils.run_bass_kernel_spmd(
        nc, in_maps, core_ids=list(range(N_CORES))
    )

    ids = np.concatenate(
        [res.results[c]["ids_u"].astype(np.int32) for c in range(N_CORES)], axis=0
    )
    q = np.concatenate(
        [res.results[c]["q"] for c in range(N_CORES)], axis=0
    )
    num = np.sum(
        [res.results[c]["lpart"].astype(np.float64).sum() for c in range(N_CORES)]
    )

    ids = np.where((pad == 1)[:, None], np.int32(-1), ids).astype(np.int32)
    ids = ids.reshape(B, T, G)
    q = q.reshape(B, T, GD).astype(np.float32)

    msum = float((1 - pad).sum())
    k = np.float32(num / msum)
    total = np.float32(2.0 * (num / msum))
    return ids, q, k, k, total
